# revision 3
# baseline (speedup 1.0000x reference)
"""Trainium2 Bass kernel v3 for DebiasNtXentLoss (B=4096, D=128, 8 cores).

Per-core decomposition over rotated znt [128, 5120] (blocks c..c+4):
  d0   (cols    0:1024): triangular per row-tile m: cols [128m, 1024).
       Processed m=4..7 first (needs only znt[512:1024] -> earlier DMA).
       Row sums on DVE; strict-upper mirror via live colsum matmuls.
  d1+d2 (cols 1024:3072): full, fused [128,2048] tiles per m; ACT accum_out
       row sums; DVE pair-adds (et12[2p]+et12[2p+1]) so colsums need only
       4 accumulating matmuls per 512-chunk (woven into the d3 window).
  d3   (cols 3072:4096): full [128,1024] per m; even m plain DVE reduce,
       odd m tensor_tensor_reduce producing the pair tile + combined row
       sum (host recovers odd = combined - even). cs-d3 reads pair tiles.
  d4   (cols 4096:5120): triangular incl. halved diagonal tiles:
       diag: 8x[128,128] matmuls -> one ACT with bias ln(1/2);
       rest strips per m; cs-d4 accumulates live on bank-wide groups.
Every sim element lands in exactly one row-acc and (mirrors) one colsum.
Host assembles rowsum[8192] from rs [128,40] + cs [1,5504] per core,
computes pos/self exactly from zn, and finishes the loss in float64.

PSUM (8 banks of 512 f32) as one [128,4096] arena:
  SA = [0:2048] (slab A / two 1024 halves), SB = [2048:4096] (slab B
  during d12; colsum accumulator banks otherwise). A PSUM bank holds ONE
  live accumulation group: every accumulation region is opened by a
  bank-wide start=True write; later writes are subset start=False adds.
"""

import numpy as np

import concourse.bacc as bacc
import concourse.bass as bass
import concourse.mybir as mybir
import concourse.tile as tile
from concourse.bass_utils import run_bass_kernel_spmd

B = 4096
D = 128
N = 2 * B
NCORES = 8
RPC = N // NCORES      # 1024
NCOL = 5 * RPC         # 5120

TEMPERATURE = 0.5
RHO = 0.1
N_NEG = N - 2
INV_T = 1.0 / TEMPERATURE
LN_HALF = float(np.log(0.5))

F32 = mybir.dt.float32
BF16 = mybir.dt.bfloat16
AF = mybir.ActivationFunctionType
AX = mybir.AxisListType
ALU = mybir.AluOpType

_CACHE = {}


def _build():
    nc = bacc.Bacc("TRN2", target_bir_lowering=False, debug=False)
    znt_dram = nc.dram_tensor("znt", [128, NCOL], BF16, kind="ExternalInput")
    rs_dram = nc.dram_tensor("rs", [128, 40], F32, kind="ExternalOutput")
    cs_dram = nc.dram_tensor("cols", [1, 5504], F32, kind="ExternalOutput")

    with tile.TileContext(nc) as tc:
        with (
            tc.tile_pool(name="big", bufs=1) as big,
            tc.tile_pool(name="small", bufs=1) as small,
            tc.tile_pool(name="psum", bufs=1, space=bass.MemorySpace.PSUM) as pp,
        ):
            znt = big.tile([128, NCOL], BF16)
            et0 = big.tile([128, 8, 1024], BF16)
            et12 = big.tile([128, 8, 2048], BF16)
            etp12 = big.tile([128, 4, 2048], BF16)
            et3 = big.tile([128, 8, 1024], BF16)
            etp3 = big.tile([128, 4, 1024], BF16)
            et4d = big.tile([128, 1024], BF16)
            et4r = big.tile([128, 7, 896], BF16)
            racc = small.tile([128, 40], F32)
            cs_sb = small.tile([1, 5504], F32)
            ones = small.tile([128, 128], BF16)
            w = small.tile([128, 1], F32)
            w2 = small.tile([128, 1], F32)
            lnh = small.tile([128, 1], F32)

            PS = pp.tile([128, 4096], F32)
            SA = PS[:, 0:2048]
            SB = PS[:, 2048:4096]

            # ---- startup: DMA descriptors, table warm, PE ramp ----
            nc.scalar.dma_start(znt[:, 0:512], znt_dram.ap()[:, 0:512])
            nc.scalar.dma_start(znt[:, 3072:4096], znt_dram.ap()[:, 3072:4096])
            nc.scalar.dma_start(znt[:, 4096:5120], znt_dram.ap()[:, 4096:5120])
            nc.sync.dma_start(znt[:, 896:1024], znt_dram.ap()[:, 896:1024])
            nc.sync.dma_start(znt[:, 512:896], znt_dram.ap()[:, 512:896])
            nc.sync.dma_start(znt[:, 1024:2048], znt_dram.ap()[:, 1024:2048])
            nc.sync.dma_start(znt[:, 2048:3072], znt_dram.ap()[:, 2048:3072])

            nc.vector.memset(ones[:], 1.0)
            nc.vector.memset(w[:], 0.0)
            nc.gpsimd.memset(lnh[:], LN_HALF)
            nc.scalar.activation(w2[:], w[:], AF.Exp)  # table warm

            for _ in range(12):
                nc.tensor.matmul(SB[:, 1536:1664], ones[:], ones[:],
                                 start=True, stop=True)

            def mm(dst, m, c0, c1):
                """slab matmuls (<=512, split at 512) dst <- znt[m]^T znt[:,c0:c1]."""
                off = 0
                while c0 < c1:
                    w_ = min(512, c1 - c0)
                    nc.tensor.matmul(
                        dst[:, off:off + w_],
                        znt[:, m * 128:(m + 1) * 128],
                        znt[:, c0:c0 + w_],
                        start=True, stop=True,
                    )
                    off += w_
                    c0 += w_

            # ================= d0: triangular, m=4..7 then 0..3 =================
            # cs-d0 accumulators (cs col j == global col base+128+j):
            #   SB[0:512)    <- cols [0:512)  from strips m=0..3
            #   SB[512:896)  <- cols [512:896) from strips m=4,5,6  (partial 1)
            #   SB[1024:1408) <- cols [512:896) from strips m=0..3  (partial 2)
            def cs_d0(m):
                # colsum matmuls for d0 strip m (reads et0[m] -> stalls PE
                # until ACT(m) done; emit AFTER the next slab fill)
                W0 = 1024 - 128 * m
                if 4 <= m < 7:
                    nc.tensor.matmul(
                        SB[:, 128 * m:896], ones[:],
                        et0[:, m, 128:W0],
                        start=(m == 4), stop=(m == 6))
                elif m < 4:
                    nc.tensor.matmul(
                        SB[:, 128 * m:512], ones[:],
                        et0[:, m, 128:640 - 128 * m],
                        start=(m == 0), stop=(m == 3))
                    nc.tensor.matmul(
                        SB[:, 1536:1920], ones[:],
                        et0[:, m, 640 - 128 * m:W0],
                        start=(m == 0), stop=(m == 3))

            D0_ORDER = (7, 4, 5, 6, 0, 1, 2, 3)
            for idx, m in enumerate(D0_ORDER):
                W0 = 1024 - 128 * m
                half = SA[:, 0:1024] if idx % 2 == 0 else SA[:, 1024:2048]
                mm(half, m, 128 * m, 1024)
                if idx > 0:
                    cs_d0(D0_ORDER[idx - 1])
                nc.scalar.activation(et0[:, m, 0:W0], half[:, 0:W0],
                                     AF.Exp, scale=INV_T)
                nc.vector.reduce_sum(racc[:, m:m + 1], et0[:, m, 0:W0],
                                     axis=AX.X)
                if m == 0:
                    nc.vector.tensor_copy(cs_sb[:, 512:896], PS[0:1, 2560:2944])
            cs_d0(3)
            nc.vector.tensor_copy(cs_sb[:, 0:512], PS[0:1, 2048:2560])
            nc.vector.tensor_copy(cs_sb[:, 4992:5376], PS[0:1, 3584:3968])

            # ================= d1+d2 fused =================
            for m in range(8):
                buf = SA if m % 2 == 0 else SB
                mm(buf, m, 1024, 3072)
                nc.scalar.activation(et12[:, m, :], buf[:],
                                     AF.Exp, scale=INV_T,
                                     accum_out=racc[:, 8 + m:9 + m])
                if m % 2 == 1:
                    nc.vector.tensor_add(etp12[:, m // 2],
                                         et12[:, m - 1], et12[:, m])

            # ================= d3 (+ cs-d12 woven) =================
            # cs-d12 chunks c=0..3 at SB[512c : 512c+512], 4 pair-accums each
            for m in range(8):
                half = SA[:, 0:1024] if m % 2 == 0 else SA[:, 1024:2048]
                mm(half, m, 3072, 4096)
                if m % 2 == 0:
                    nc.scalar.activation(et3[:, m, :], half[:],
                                         AF.Exp, scale=INV_T)
                    nc.vector.reduce_sum(racc[:, 16 + m:17 + m], et3[:, m, :],
                                         axis=AX.X)
                else:
                    nc.scalar.activation(et3[:, m, :], half[:],
                                         AF.Exp, scale=INV_T)
                    nc.vector.reduce_sum(racc[:, 16 + m:17 + m], et3[:, m, :],
                                         axis=AX.X)
                    nc.vector.tensor_add(etp3[:, m // 2],
                                         et3[:, m - 1], et3[:, m])
                c = m % 4
                for p in ((0, 1) if m < 4 else (2, 3)):
                    nc.tensor.matmul(
                        SB[:, 512 * c:512 * (c + 1)], ones[:],
                        etp12[:, p, 512 * c:512 * (c + 1)],
                        start=(p == 0), stop=(p == 3))
                if m >= 5:
                    c_done = m - 5  # chunk c closes at m = 4 + c
                    nc.vector.tensor_copy(
                        cs_sb[:, 896 + 512 * c_done:1408 + 512 * c_done],
                        PS[0:1, 2048 + 512 * c_done:2560 + 512 * c_done])

            # ================= d4 diag (halved) =================
            dhalf = SA[:, 0:1024]
            for m in range(8):
                nc.tensor.matmul(
                    dhalf[:, 128 * m:128 * (m + 1)],
                    znt[:, 128 * m:128 * (m + 1)],
                    znt[:, 4096 + 128 * m:4096 + 128 * (m + 1)],
                    start=True, stop=True)
            nc.scalar.activation(et4d[:], dhalf[:], AF.Exp,
                                 bias=lnh[:], scale=INV_T)
            nc.vector.reduce_sum(
                racc[:, 24:32],
                et4d[:].rearrange("p (i x) -> p i x", i=8), axis=AX.X)
            # last cs-d12 chunk copy
            nc.vector.tensor_copy(cs_sb[:, 2432:2944], PS[0:1, 3584:4096])

            def cs_d4_strip(m):
                # strip m covers cs cols [128(m+1), 1024); bank 4's last
                # writer is strip m=2 (piece ending 512), bank 5's m=6.
                a, b = 128 * (m + 1), 1024
                while a < b:
                    w_ = min(512 - (a % 512), b - a)
                    sa_ = a - 128 * (m + 1)
                    nc.tensor.matmul(
                        SB[:, a:a + w_], ones[:], et4r[:, m, sa_:sa_ + w_],
                        start=False,
                        stop=((m == 2 and a + w_ == 512)
                              or (m == 6 and a + w_ == 1024)))
                    a += w_

            # ================= d4 rest strips (+ cs-d3 woven) =================
            for m in range(7):
                W4 = 896 - 128 * m
                half = SA[:, 0:1024] if m % 2 == 1 else SA[:, 1024:2048]
                mm(half, m, 4096 + 128 * (m + 1), 5120)
                if m == 0:
                    # cs-d4 init: bank-wide groups over SB[0:1024] (stalls
                    # on the et4d ACT; next slab is already queued ahead)
                    nc.tensor.matmul(SB[:, 0:512], ones[:], et4d[:, 0:512],
                                     start=True, stop=False)
                    nc.tensor.matmul(SB[:, 512:1024], ones[:],
                                     et4d[:, 512:1024],
                                     start=True, stop=False)
                else:
                    cs_d4_strip(m - 1)
                nc.scalar.activation(et4r[:, m, 0:W4], half[:, 0:W4],
                                     AF.Exp, scale=INV_T,
                                     accum_out=racc[:, 32 + m:33 + m])
                # cs-d3: 2 chunks x 4 pair-accums at SB[1024:1536),[1536:2048)
                if m < 4:
                    c = m // 2
                    for p in ((0, 1) if m % 2 == 0 else (2, 3)):
                        nc.tensor.matmul(
                            SB[:, 1024 + 512 * c:1536 + 512 * c], ones[:],
                            etp3[:, p, 512 * c:512 * (c + 1)],
                            start=(p == 0), stop=(p == 3))
                if m == 2:
                    nc.vector.tensor_copy(cs_sb[:, 2944:3456],
                                          PS[0:1, 3072:3584])
                if m == 3:
                    # cs-d4 bank 4 closed by strip m=2 -> copy early
                    nc.vector.tensor_copy(cs_sb[:, 3968:4480],
                                          PS[0:1, 2048:2560])
                if m == 4:
                    nc.vector.tensor_copy(cs_sb[:, 3456:3968],
                                          PS[0:1, 3584:4096])
                    # early output DMA: everything copied so far
                    nc.sync.dma_start(rs_dram.ap()[:, 0:24], racc[:, 0:24])
                    nc.sync.dma_start(cs_dram.ap()[:, 0:4480], cs_sb[:, 0:4480])
            cs_d4_strip(6)
            nc.vector.tensor_copy(cs_sb[:, 4480:4992], PS[0:1, 2560:3072])

            nc.sync.dma_start(rs_dram.ap()[:, 24:40], racc[:, 24:40])
            nc.sync.dma_start(cs_dram.ap()[:, 4480:5504], cs_sb[:, 4480:5504])

    nc.compile()
    return nc


def _get_nc():
    if "nc" not in _CACHE:
        _CACHE["nc"] = _build()
    return _CACHE["nc"]


def _prep_inputs(z_i, z_j):
    import ml_dtypes

    z = np.concatenate(
        [np.asarray(z_i, np.float32), np.asarray(z_j, np.float32)], axis=0
    )
    zn = z / np.maximum(
        np.sqrt((z * z).sum(axis=1, keepdims=True, dtype=np.float32)), 1e-8
    ).astype(np.float32)
    znt = np.ascontiguousarray(zn.T).astype(ml_dtypes.bfloat16)  # [128, 8192]
    in_maps = []
    for c in range(NCORES):
        znt_c = np.roll(znt, -c * RPC, axis=1)[:, :NCOL]
        in_maps.append({"znt": np.ascontiguousarray(znt_c)})
    return in_maps, zn


def kernel(z_i, z_j, _want_results=False, **run_kwargs):
    nc = _get_nc()
    in_maps, zn = _prep_inputs(z_i, z_j)
    out = run_bass_kernel_spmd(
        nc, in_maps, core_ids=list(range(NCORES)), **run_kwargs
    )
    rowsum = np.zeros(N, dtype=np.float64)
    idx = np.arange(2048)
    for c in range(NCORES):
        rs = out.results[c]["rs"].astype(np.float64)       # [128, 40]
        cs = out.results[c]["cols"][0].astype(np.float64)  # [5504]
        base = c * RPC
        for m in range(8):
            rows = slice(base + 128 * m, base + 128 * (m + 1))
            tot = rs[:, m] + rs[:, 8 + m] + rs[:, 16 + m] + rs[:, 24 + m]
            if m < 7:
                tot = tot + rs[:, 32 + m]
            rowsum[rows] += tot
        # colsum mirrors
        rowsum[base + 128: base + 1024] += cs[0:896]
        rowsum[base + 640: base + 1024] += cs[4992:5376]   # cs-d0 partial 2
        rowsum[(base + 1024 + idx) % N] += cs[896:2944]
        rowsum[(base + 3072 + idx[:1024]) % N] += cs[2944:3968]
        rowsum[(base + 4096 + idx[:1024]) % N] += cs[3968:4992]

    zn64 = zn.astype(np.float64)
    pos = np.exp(INV_T * np.sum(zn64 * np.roll(zn64, -B, axis=0), axis=1))
    slf = np.exp(INV_T * np.sum(zn64 * zn64, axis=1))
    neg = rowsum - slf - pos
    ng = (-RHO * N_NEG * pos + neg) / (1.0 - RHO)
    ng = np.maximum(ng, N_NEG * np.exp(-1.0 / TEMPERATURE))
    losses = np.log(pos + ng) - np.log(pos)
    loss = np.float32(losses.mean())
    if _want_results:
        return loss, out
    return loss


# revision 4
# speedup vs baseline: 1.0288x; 1.0288x over previous
"""Trainium2 Bass kernel v3 for DebiasNtXentLoss (B=4096, D=128, 8 cores).

Per-core decomposition over rotated znt [128, 5120] (blocks c..c+4):
  d0   (cols    0:1024): triangular per row-tile m: cols [128m, 1024).
       Processed m=4..7 first (needs only znt[512:1024] -> earlier DMA).
       Row sums on DVE; strict-upper mirror via live colsum matmuls.
  d1+d2 (cols 1024:3072): full, fused [128,2048] tiles per m; ACT accum_out
       row sums; DVE pair-adds (et12[2p]+et12[2p+1]) so colsums need only
       4 accumulating matmuls per 512-chunk (woven into the d3 window).
  d3   (cols 3072:4096): full [128,1024] per m; even m plain DVE reduce,
       odd m tensor_tensor_reduce producing the pair tile + combined row
       sum (host recovers odd = combined - even). cs-d3 reads pair tiles.
  d4   (cols 4096:5120): triangular incl. halved diagonal tiles:
       diag: 8x[128,128] matmuls -> one ACT with bias ln(1/2);
       rest strips per m; cs-d4 accumulates live on bank-wide groups.
Every sim element lands in exactly one row-acc and (mirrors) one colsum.
Host assembles rowsum[8192] from rs [128,40] + cs [1,5504] per core,
computes pos/self exactly from zn, and finishes the loss in float64.

PSUM (8 banks of 512 f32) as one [128,4096] arena:
  SA = [0:2048] (slab A / two 1024 halves), SB = [2048:4096] (slab B
  during d12; colsum accumulator banks otherwise). A PSUM bank holds ONE
  live accumulation group: every accumulation region is opened by a
  bank-wide start=True write; later writes are subset start=False adds.
"""

import numpy as np

import concourse.bacc as bacc
import concourse.bass as bass
import concourse.mybir as mybir
import concourse.tile as tile
from concourse.bass_utils import run_bass_kernel_spmd

B = 4096
D = 128
N = 2 * B
NCORES = 8
RPC = N // NCORES      # 1024
NCOL = 5 * RPC         # 5120

TEMPERATURE = 0.5
RHO = 0.1
N_NEG = N - 2
INV_T = 1.0 / TEMPERATURE
LN_HALF = float(np.log(0.5))

F32 = mybir.dt.float32
BF16 = mybir.dt.bfloat16
AF = mybir.ActivationFunctionType
AX = mybir.AxisListType
ALU = mybir.AluOpType

_CACHE = {}


def _build():
    nc = bacc.Bacc("TRN2", target_bir_lowering=False, debug=False)
    znt_dram = nc.dram_tensor("znt", [128, NCOL], BF16, kind="ExternalInput")
    rs_dram = nc.dram_tensor("rs", [128, 40], F32, kind="ExternalOutput")
    cs_dram = nc.dram_tensor("cols", [1, 5504], F32, kind="ExternalOutput")

    with tile.TileContext(nc) as tc:
        with (
            tc.tile_pool(name="big", bufs=1) as big,
            tc.tile_pool(name="small", bufs=1) as small,
            tc.tile_pool(name="psum", bufs=1, space=bass.MemorySpace.PSUM) as pp,
        ):
            znt = big.tile([128, NCOL], BF16)
            et0 = big.tile([128, 8, 1024], BF16)
            et12 = big.tile([128, 8, 2048], BF16)
            etp12 = big.tile([128, 4, 2048], BF16)
            et3 = big.tile([128, 8, 1024], BF16)
            etp3 = big.tile([128, 4, 1024], BF16)
            et4d = big.tile([128, 1024], BF16)
            et4r = big.tile([128, 7, 896], BF16)
            racc = small.tile([128, 40], F32)
            cs_sb = small.tile([1, 5504], F32)
            ones = small.tile([128, 128], BF16)
            w = small.tile([128, 1], F32)
            w2 = small.tile([128, 1], F32)
            lnh = small.tile([128, 1], F32)

            PS = pp.tile([128, 4096], F32)
            SA = PS[:, 0:2048]
            SB = PS[:, 2048:4096]

            # ---- startup: DMA descriptors, table warm, PE ramp ----
            nc.scalar.dma_start(znt[:, 3072:4096], znt_dram.ap()[:, 3072:4096])
            nc.scalar.dma_start(znt[:, 4096:5120], znt_dram.ap()[:, 4096:5120])
            nc.sync.dma_start(znt[:, 896:1024], znt_dram.ap()[:, 896:1024])
            nc.sync.dma_start(znt[:, 512:896], znt_dram.ap()[:, 512:896])
            nc.sync.dma_start(znt[:, 0:512], znt_dram.ap()[:, 0:512])
            nc.sync.dma_start(znt[:, 1024:2048], znt_dram.ap()[:, 1024:2048])
            nc.sync.dma_start(znt[:, 2048:3072], znt_dram.ap()[:, 2048:3072])

            nc.vector.memset(ones[:], 1.0)
            nc.vector.memset(w[:], 0.0)
            nc.gpsimd.memset(lnh[:], LN_HALF)
            nc.scalar.activation(w2[:], w[:], AF.Exp)  # table warm

            for _ in range(12):
                nc.tensor.matmul(SB[:, 1536:1664], ones[:], ones[:],
                                 start=True, stop=True)

            def mm(dst, m, c0, c1):
                """slab matmuls (<=512, split at 512) dst <- znt[m]^T znt[:,c0:c1]."""
                off = 0
                while c0 < c1:
                    w_ = min(512, c1 - c0)
                    nc.tensor.matmul(
                        dst[:, off:off + w_],
                        znt[:, m * 128:(m + 1) * 128],
                        znt[:, c0:c0 + w_],
                        start=True, stop=True,
                    )
                    off += w_
                    c0 += w_

            # ================= d0: triangular, m=4..7 then 0..3 =================
            # cs-d0 accumulators (cs col j == global col base+128+j):
            #   SB[0:512)    <- cols [0:512)  from strips m=0..3
            #   SB[512:896)  <- cols [512:896) from strips m=4,5,6  (partial 1)
            #   SB[1024:1408) <- cols [512:896) from strips m=0..3  (partial 2)
            def cs_d0(m):
                # colsum matmuls for d0 strip m (reads et0[m] -> stalls PE
                # until ACT(m) done; emit AFTER the next slab fill)
                W0 = 1024 - 128 * m
                if 4 <= m < 7:
                    nc.tensor.matmul(
                        SB[:, 128 * m:896], ones[:],
                        et0[:, m, 128:W0],
                        start=(m == 4), stop=(m == 6))
                elif m < 4:
                    nc.tensor.matmul(
                        SB[:, 128 * m:512], ones[:],
                        et0[:, m, 128:640 - 128 * m],
                        start=(m == 0), stop=(m == 3))
                    nc.tensor.matmul(
                        SB[:, 1536:1920], ones[:],
                        et0[:, m, 640 - 128 * m:W0],
                        start=(m == 0), stop=(m == 3))

            D0_ORDER = (7, 4, 5, 6, 0, 1, 2, 3)
            for idx, m in enumerate(D0_ORDER):
                W0 = 1024 - 128 * m
                half = SA[:, 0:1024] if idx % 2 == 0 else SA[:, 1024:2048]
                mm(half, m, 128 * m, 1024)
                if idx > 0:
                    cs_d0(D0_ORDER[idx - 1])
                nc.scalar.activation(et0[:, m, 0:W0], half[:, 0:W0],
                                     AF.Exp, scale=INV_T)
                nc.vector.reduce_sum(racc[:, m:m + 1], et0[:, m, 0:W0],
                                     axis=AX.X)
                if m == 0:
                    nc.vector.tensor_copy(cs_sb[:, 512:896], PS[0:1, 2560:2944])
            cs_d0(3)
            nc.vector.tensor_copy(cs_sb[:, 0:512], PS[0:1, 2048:2560])
            nc.vector.tensor_copy(cs_sb[:, 4992:5376], PS[0:1, 3584:3968])

            # ================= d1+d2 fused =================
            for m in range(8):
                buf = SA if m % 2 == 0 else SB
                mm(buf, m, 1024, 3072)
                nc.scalar.activation(et12[:, m, :], buf[:],
                                     AF.Exp, scale=INV_T,
                                     accum_out=racc[:, 8 + m:9 + m])
                if m % 2 == 1:
                    nc.vector.tensor_add(etp12[:, m // 2],
                                         et12[:, m - 1], et12[:, m])

            # ================= d3 (+ cs-d12 woven) =================
            # cs-d12 chunks c=0..3 at SB[512c : 512c+512], 4 pair-accums each
            for m in range(8):
                half = SA[:, 0:1024] if m % 2 == 0 else SA[:, 1024:2048]
                mm(half, m, 3072, 4096)
                if m % 2 == 0:
                    nc.scalar.activation(et3[:, m, :], half[:],
                                         AF.Exp, scale=INV_T)
                    nc.vector.reduce_sum(racc[:, 16 + m:17 + m], et3[:, m, :],
                                         axis=AX.X)
                else:
                    nc.scalar.activation(et3[:, m, :], half[:],
                                         AF.Exp, scale=INV_T)
                    nc.vector.reduce_sum(racc[:, 16 + m:17 + m], et3[:, m, :],
                                         axis=AX.X)
                    nc.vector.tensor_add(etp3[:, m // 2],
                                         et3[:, m - 1], et3[:, m])
                c = m % 4
                for p in ((0, 1) if m < 4 else (2, 3)):
                    nc.tensor.matmul(
                        SB[:, 512 * c:512 * (c + 1)], ones[:],
                        etp12[:, p, 512 * c:512 * (c + 1)],
                        start=(p == 0), stop=(p == 3))
                if m >= 5:
                    c_done = m - 5  # chunk c closes at m = 4 + c
                    nc.vector.tensor_copy(
                        cs_sb[:, 896 + 512 * c_done:1408 + 512 * c_done],
                        PS[0:1, 2048 + 512 * c_done:2560 + 512 * c_done])

            # ================= d4 diag (halved) =================
            dhalf = SA[:, 0:1024]
            for m in range(8):
                nc.tensor.matmul(
                    dhalf[:, 128 * m:128 * (m + 1)],
                    znt[:, 128 * m:128 * (m + 1)],
                    znt[:, 4096 + 128 * m:4096 + 128 * (m + 1)],
                    start=True, stop=True)
            nc.scalar.activation(et4d[:], dhalf[:], AF.Exp,
                                 bias=lnh[:], scale=INV_T)
            nc.vector.reduce_sum(
                racc[:, 24:32],
                et4d[:].rearrange("p (i x) -> p i x", i=8), axis=AX.X)
            # last cs-d12 chunk copy
            nc.vector.tensor_copy(cs_sb[:, 2432:2944], PS[0:1, 3584:4096])

            def cs_d4_strip(m):
                # strip m covers cs cols [128(m+1), 1024); bank 4's last
                # writer is strip m=2 (piece ending 512), bank 5's m=6.
                a, b = 128 * (m + 1), 1024
                while a < b:
                    w_ = min(512 - (a % 512), b - a)
                    sa_ = a - 128 * (m + 1)
                    nc.tensor.matmul(
                        SB[:, a:a + w_], ones[:], et4r[:, m, sa_:sa_ + w_],
                        start=False,
                        stop=((m == 2 and a + w_ == 512)
                              or (m == 6 and a + w_ == 1024)))
                    a += w_

            # ================= d4 rest strips (+ cs-d3 woven) =================
            for m in range(7):
                W4 = 896 - 128 * m
                half = SA[:, 0:1024] if m % 2 == 1 else SA[:, 1024:2048]
                mm(half, m, 4096 + 128 * (m + 1), 5120)
                if m == 0:
                    # cs-d4 init: bank-wide groups over SB[0:1024] (stalls
                    # on the et4d ACT; next slab is already queued ahead)
                    nc.tensor.matmul(SB[:, 0:512], ones[:], et4d[:, 0:512],
                                     start=True, stop=False)
                    nc.tensor.matmul(SB[:, 512:1024], ones[:],
                                     et4d[:, 512:1024],
                                     start=True, stop=False)
                else:
                    cs_d4_strip(m - 1)
                nc.scalar.activation(et4r[:, m, 0:W4], half[:, 0:W4],
                                     AF.Exp, scale=INV_T,
                                     accum_out=racc[:, 32 + m:33 + m])
                # cs-d3: 2 chunks x 4 pair-accums at SB[1024:1536),[1536:2048)
                if m < 4:
                    c = m // 2
                    for p in ((0, 1) if m % 2 == 0 else (2, 3)):
                        nc.tensor.matmul(
                            SB[:, 1024 + 512 * c:1536 + 512 * c], ones[:],
                            etp3[:, p, 512 * c:512 * (c + 1)],
                            start=(p == 0), stop=(p == 3))
                if m == 2:
                    nc.vector.tensor_copy(cs_sb[:, 2944:3456],
                                          PS[0:1, 3072:3584])
                if m == 3:
                    # cs-d4 bank 4 closed by strip m=2 -> copy early
                    nc.vector.tensor_copy(cs_sb[:, 3968:4480],
                                          PS[0:1, 2048:2560])
                if m == 4:
                    nc.vector.tensor_copy(cs_sb[:, 3456:3968],
                                          PS[0:1, 3584:4096])
                    # early output DMA: everything copied so far
                    nc.sync.dma_start(rs_dram.ap()[:, 0:24], racc[:, 0:24])
                    nc.sync.dma_start(cs_dram.ap()[:, 0:4480], cs_sb[:, 0:4480])
            cs_d4_strip(6)
            nc.vector.tensor_copy(cs_sb[:, 4480:4992], PS[0:1, 2560:3072])

            nc.sync.dma_start(rs_dram.ap()[:, 24:40], racc[:, 24:40])
            nc.sync.dma_start(cs_dram.ap()[:, 4480:5504], cs_sb[:, 4480:5504])

    nc.compile()
    return nc


def _get_nc():
    if "nc" not in _CACHE:
        _CACHE["nc"] = _build()
    return _CACHE["nc"]


def _prep_inputs(z_i, z_j):
    import ml_dtypes

    z = np.concatenate(
        [np.asarray(z_i, np.float32), np.asarray(z_j, np.float32)], axis=0
    )
    zn = z / np.maximum(
        np.sqrt((z * z).sum(axis=1, keepdims=True, dtype=np.float32)), 1e-8
    ).astype(np.float32)
    znt = np.ascontiguousarray(zn.T).astype(ml_dtypes.bfloat16)  # [128, 8192]
    in_maps = []
    for c in range(NCORES):
        znt_c = np.roll(znt, -c * RPC, axis=1)[:, :NCOL]
        in_maps.append({"znt": np.ascontiguousarray(znt_c)})
    return in_maps, zn


def kernel(z_i, z_j, _want_results=False, **run_kwargs):
    nc = _get_nc()
    in_maps, zn = _prep_inputs(z_i, z_j)
    out = run_bass_kernel_spmd(
        nc, in_maps, core_ids=list(range(NCORES)), **run_kwargs
    )
    rowsum = np.zeros(N, dtype=np.float64)
    idx = np.arange(2048)
    for c in range(NCORES):
        rs = out.results[c]["rs"].astype(np.float64)       # [128, 40]
        cs = out.results[c]["cols"][0].astype(np.float64)  # [5504]
        base = c * RPC
        for m in range(8):
            rows = slice(base + 128 * m, base + 128 * (m + 1))
            tot = rs[:, m] + rs[:, 8 + m] + rs[:, 16 + m] + rs[:, 24 + m]
            if m < 7:
                tot = tot + rs[:, 32 + m]
            rowsum[rows] += tot
        # colsum mirrors
        rowsum[base + 128: base + 1024] += cs[0:896]
        rowsum[base + 640: base + 1024] += cs[4992:5376]   # cs-d0 partial 2
        rowsum[(base + 1024 + idx) % N] += cs[896:2944]
        rowsum[(base + 3072 + idx[:1024]) % N] += cs[2944:3968]
        rowsum[(base + 4096 + idx[:1024]) % N] += cs[3968:4992]

    zn64 = zn.astype(np.float64)
    pos = np.exp(INV_T * np.sum(zn64 * np.roll(zn64, -B, axis=0), axis=1))
    slf = np.exp(INV_T * np.sum(zn64 * zn64, axis=1))
    neg = rowsum - slf - pos
    ng = (-RHO * N_NEG * pos + neg) / (1.0 - RHO)
    ng = np.maximum(ng, N_NEG * np.exp(-1.0 / TEMPERATURE))
    losses = np.log(pos + ng) - np.log(pos)
    loss = np.float32(losses.mean())
    if _want_results:
        return loss, out
    return loss
